# revision 40
# baseline (speedup 1.0000x reference)
"""Trainium2 Bass kernel v3: MultiHeadAttention with rel-pos bias via
host-LUT bias tiles + per-head score layout + PE array tiling.

Problem: B=4, S=2048, D=256, H=8, d_k=32.  8 cores = (batch, query-half);
each core: 8 heads x 1024 q x 2048 k.

v2 (one-hot-plane matmuls in a packed (h,k16) layout) was PE-bound
(~924us MATMUL: 8 matmuls of 512 free per 128x512 score tile) plus 75MB
of 9x-replicated rel_pos DMA (~104 GB/s achieved -> DMA co-critical).

v3: the bias bias[k,q,h] = ef[rpF,h] + eb[rpN,h] takes only 100 values
per head, so the host folds it through a 100x8 LUT into int8 tiles laid
out exactly as the SBUF tiles consume them (16MB/core, contiguous 512KB
DMAs).  Device uses a per-head score layout [128 k, 512 q]:
  scores = K_h^T Q_h / s        1 matmul, contraction 32, row-tiled:
                                4 heads run CONCURRENT in the PE array
                                (tile_position=(32m,0), 4 PSUM banks)
  att_pre = scores + q8         DVE add (int8 bias), out fp16 SBUF
  att     = exp(s*att_pre)      ACT, fp16 in/out (2x rate), scale=s
  psAV   += Vaug_h^T att        1 matmul; Vaug has a ones column so the
                                denominator rides along as out row 32;
                                2 heads/bank at offsets {0,64} run
                                concurrent via col tiling
All KQ packs are emitted before all AV pairs within a k-tile so the PE
stream has only 2 tiling-mode transitions per k-tile -- interleaving
row-mode and col-mode matmuls every ~2 instructions intermittently hard
-crashed an exec unit (NRT_EXEC_UNIT_UNRECOVERABLE, ~20% of runs).

Measured: 247us HW exec (vs 784us v2 baseline), rel err 0.0081
(dominated by the int8 bias quantization; budget 2e-2).
"""

import sys

if "/opt/trn_rl_repo" not in sys.path:
    sys.path.insert(0, "/opt/trn_rl_repo")

import numpy as np

import concourse.bass as bass
import concourse.mybir as mybir
from concourse import bacc
from concourse.tile import TileContext
from concourse.bass_utils import run_bass_kernel_spmd

B, S, D, H = 4, 2048, 256, 8
D_K = D // H
QH = S // 2
N_CORES = 8
KT = S // 128           # 16 k-tiles of 128
FP32 = mybir.dt.float32
FP16 = mybir.dt.float16
BF16 = mybir.dt.bfloat16
INT8 = mybir.dt.int8

BIAS_INT8 = True        # True: int8 bias tiles (16MB/core); False: fp16 (32MB)
BIAS_DT = INT8 if BIAS_INT8 else FP16


def _build():
    nc = bacc.Bacc("TRN2", target_bir_lowering=False, debug=False)

    qT = nc.dram_tensor("qT", [D, QH], FP16, kind="ExternalInput").ap()
    kT = nc.dram_tensor("kT", [D, S], FP16, kind="ExternalInput").ap()
    vT = nc.dram_tensor("vT", [D, S], FP16, kind="ExternalInput").ap()
    wqT = nc.dram_tensor("wqT", [D, D], FP16, kind="ExternalInput").ap()
    wkT = nc.dram_tensor("wkT", [D, D], FP16, kind="ExternalInput").ap()
    wvT = nc.dram_tensor("wvT", [D, D], FP16, kind="ExternalInput").ap()
    wpT = nc.dram_tensor("wpT", [D, D], FP16, kind="ExternalInput").ap()
    bqs = nc.dram_tensor("bqs", [128, 2], FP32, kind="ExternalInput").ap()
    bps = nc.dram_tensor("bps", [128, 2], FP32, kind="ExternalInput").ap()
    ldn = nc.dram_tensor("ldn", [8, 256], FP16, kind="ExternalInput").ap()
    svec = nc.dram_tensor("svec", [128, 2], FP32, kind="ExternalInput").ap()
    # bias tiles pre-packed host-side: row block (qc*16+kt)*128 .. +128 is
    # one SBUF tile [128 k, (h, 512 q)].  Heads 0-5 ship int8 (added on the
    # DVE); heads 6-7 ship fp16 (added on the PE via an identity-matmul
    # accumulate, relieving the DVE which is the critical engine).
    biasT = nc.dram_tensor("biasT", [32 * 128, 6 * 512], BIAS_DT,
                           kind="ExternalInput").ap()
    biasF = nc.dram_tensor("biasF", [32 * 128, 2 * 512], FP16,
                           kind="ExternalInput").ap()
    idt = nc.dram_tensor("idt", [128, 128], FP16, kind="ExternalInput").ap()
    outT = nc.dram_tensor("outT", [D, QH], FP32, kind="ExternalOutput").ap()

    with TileContext(nc) as tc:
        _emit(nc, tc, locals())
    nc.compile()
    return nc


def _emit(nc, tc, t):
    qT, kT, vT = t["qT"], t["kT"], t["vT"]
    wqT, wkT, wvT, wpT = t["wqT"], t["wkT"], t["wvT"], t["wpT"]
    bqs, bps, ldn, svec = t["bqs"], t["bps"], t["ldn"], t["svec"]
    biasT, biasF, idt, outT = t["biasT"], t["biasF"], t["idt"], t["outT"]
    Exp = mybir.ActivationFunctionType.Exp
    Ident = mybir.ActivationFunctionType.Identity
    AOT = mybir.AluOpType

    import contextlib
    ctx = contextlib.ExitStack()
    with ctx:
        singles = ctx.enter_context(tc.tile_pool(name="singles", bufs=1))
        stage = ctx.enter_context(tc.tile_pool(name="stage", bufs=2))
        biasp = ctx.enter_context(tc.tile_pool(name="biasp", bufs=4))
        prep = ctx.enter_context(tc.tile_pool(name="prep", bufs=3))
        attp = ctx.enter_context(tc.tile_pool(name="attp", bufs=2))
        # two 2-bank score tiles [128, 1024] rotate so the PE fills one
        # while the DVE drains the other; all other psum users slice them
        psS = ctx.enter_context(tc.tile_pool(name="psS", bufs=2, space="PSUM"))
        psAV = ctx.enter_context(tc.tile_pool(name="psAV", bufs=1, space="PSUM"))

        # ---- constants ----
        c_sb = {}
        for name, ap, shp, dt in (
            ("bqs", bqs, [128, 2], FP32), ("bps", bps, [128, 2], FP32),
            ("ldn", ldn, [8, 256], FP16), ("svec", svec, [128, 2], FP32),
            ("idt", idt, [128, 128], FP16),
        ):
            tl = singles.tile(shp, dt, name=name, tag=name)
            nc.sync.dma_start(out=tl, in_=ap)
            c_sb[name] = tl

        # ---- weights: [din-group][128, 256] ----
        w_sb = {}
        for name, ap in (("wq", wqT), ("wk", wkT), ("wv", wvT), ("wp", wpT)):
            for g in range(2):
                tl = singles.tile([128, D], FP16, name=f"w_{name}{g}", tag=f"w_{name}{g}")
                nc.sync.dma_start(out=tl, in_=ap[g * 128:(g + 1) * 128, :])
                w_sb[name, g] = tl

        # ---- raw inputs resident ----
        xin = {}
        for name, ap, width in (("q", qT, QH), ("k", kT, S), ("v", vT, S)):
            for g in range(2):
                tl = singles.tile([128, width], FP16, name=f"{name}in{g}", tag=f"{name}in{g}")
                nc.sync.dma_start(out=tl, in_=ap[g * 128:(g + 1) * 128, :])
                xin[name, g] = tl

        # ---- Q/K projections -> QTs/KTs [g][128, *] fp16 (dout-major) ----
        QTs = [singles.tile([128, QH], FP16, name=f"QTs{g}", tag=f"QTs{g}") for g in range(2)]
        KTs = [singles.tile([128, S], FP16, name=f"KTs{g}", tag=f"KTs{g}") for g in range(2)]
        for dst, src, wname, bias_name, width in (
            (QTs, "q", "wq", "bqs", QH),
            (KTs, "k", "wk", None, S),
        ):
            for c0 in range(0, width, 512):
                big = psS.tile([128, 1024], FP32, name="proj", tag="scores")
                for g in range(2):
                    ps = big[:, g * 512:g * 512 + 512]
                    for dg in range(2):
                        nc.tensor.matmul(
                            ps, w_sb[wname, dg][:, g * 128:(g + 1) * 128],
                            xin[src, dg][:, c0:c0 + 512],
                            start=(dg == 0), stop=(dg == 1))
                    # alternate psum evacuation between ACT and DVE so the
                    # prologue isn't serialized on one engine
                    if bias_name:
                        if g == 0:
                            nc.scalar.activation(
                                dst[g][:, c0:c0 + 512], ps, Ident,
                                bias=c_sb[bias_name][:, g:g + 1])
                        else:
                            nc.vector.tensor_scalar(
                                out=dst[g][:, c0:c0 + 512], in0=ps,
                                scalar1=c_sb[bias_name][:, g:g + 1],
                                scalar2=None, op0=AOT.add)
                    else:
                        if g == 0:
                            nc.scalar.copy(dst[g][:, c0:c0 + 512], ps)
                        else:
                            nc.vector.tensor_scalar_add(
                                dst[g][:, c0:c0 + 512], ps, 0.0)

        # ---- Vaug[kt] [128 s, 264=(h: 32 dv + one)] fp16 ----
        vaug = []
        for kt2 in range(KT // 2):
            big = psS.tile([128, 1024], FP32, name="vproj", tag="scores")
            for ki in range(2):
                kt = kt2 * 2 + ki
                vt = singles.tile([128, 264], FP16, name=f"vaug{kt}", tag=f"vaug{kt}")
                ones_ap = bass.AP(tensor=vt.tensor, offset=vt.offset + 32,
                                  ap=[list(vt.ap[0]), [33, 8]])
                nc.gpsimd.memset(ones_ap, 1.0)
                vps = big[:, ki * 512:ki * 512 + 512]
                for dg in range(2):
                    nc.tensor.matmul(
                        vps[:, 0:256], xin["v", dg][:, kt * 128:(kt + 1) * 128],
                        w_sb["wv", dg], start=(dg == 0), stop=(dg == 1))
                dst_ap = bass.AP(tensor=vt.tensor, offset=vt.offset,
                                 ap=[list(vt.ap[0]), [33, 8], [1, 32]])
                src_ap = bass.AP(tensor=vps.tensor, offset=vps.offset,
                                 ap=[list(vps.ap[0]), [32, 8], [1, 32]])
                if ki == 0:
                    nc.scalar.copy(dst_ap, src_ap)
                else:
                    nc.vector.tensor_scalar_add(dst_ap, src_ap, 0.0)
                vaug.append(vt)

        # ---- main loop ----
        for qc in range(2):
            q0 = qc * 512
            pav = [psAV.tile([128, 512], FP32, name=f"psAV{j}", tag=f"psAV{j}")
                   for j in range(4)]
            for kt in range(KT):
                bt = biasp.tile([128, 6 * 512], BIAS_DT, name="bt", tag="bt")
                btf = biasp.tile([128, 2 * 512], FP16, name="btf", tag="btf")
                r0 = (qc * KT + kt) * 128
                for quad in range(3):
                    eng = (nc.sync, nc.gpsimd, nc.sync)[quad]
                    eng.dma_start(
                        out=bt[:, quad * 1024:(quad + 1) * 1024],
                        in_=biasT[r0:r0 + 128, quad * 1024:(quad + 1) * 1024])
                nc.gpsimd.dma_start(out=btf, in_=biasF[r0:r0 + 128, :])
                # Emit all KQ packs first, all AV pairs last: the PE stream
                # then has few tiling-mode transitions per kt.  Packs 0-2 add
                # their int8 bias on the DVE (the critical engine); pack 3
                # accumulates fp16 bias in PSUM via identity matmuls and is
                # exp'd straight from PSUM, saving one DVE pass per kt.
                att = []
                apre = None
                big3 = None
                for p in range(4):
                    g = p // 2
                    big = psS.tile([128, 1024], FP32, name="scores", tag="scores")
                    for m in range(2):
                        r = 32 * ((2 * p + m) % 4)
                        nc.tensor.matmul(
                            big[:, m * 512:m * 512 + 512],
                            KTs[g][r:r + 32, kt * 128:(kt + 1) * 128],
                            QTs[g][r:r + 32, q0:q0 + 512],
                            start=True, stop=(p < 3),
                            tile_position=(r, 0))
                    if p < 3:
                        if p % 2 == 0:
                            apre = attp.tile([128, 2048], FP16, name="att_pre",
                                             tag="att_pre")
                        hp = (p % 2) * 1024
                        nc.vector.tensor_tensor(
                            out=apre[:, hp:hp + 1024], in0=big,
                            in1=bt[:, p * 1024:(p + 1) * 1024], op=AOT.add)
                    else:
                        big3 = big
                        for m in range(2):
                            nc.tensor.matmul(
                                big[:, m * 512:m * 512 + 512], c_sb["idt"],
                                btf[:, m * 512:m * 512 + 512],
                                start=False, stop=True)
                    if p == 1:
                        # -4 shift keeps exp in fp16 range (logit tail ~12.5 >
                        # ln 65504); cancels between num. and denominator.
                        at = prep.tile([128, 2048], FP16, name="att", tag="att")
                        nc.scalar.activation(at, apre, Exp,
                                             bias=c_sb["svec"][:, 1:2],
                                             scale=c_sb["svec"][:, 0:1])
                        att.append(at)
                at2 = prep.tile([128, 2048], FP16, name="att", tag="att")
                nc.scalar.activation(at2[:, 0:1024], apre[:, 0:1024], Exp,
                                     bias=c_sb["svec"][:, 1:2],
                                     scale=c_sb["svec"][:, 0:1])
                nc.scalar.activation(at2[:, 1024:2048], big3, Exp,
                                     bias=c_sb["svec"][:, 1:2],
                                     scale=c_sb["svec"][:, 0:1])
                att.append(at2)
                # AV+den: col-packed pairs, 2 heads into bank p at {0, 64}
                for p in range(4):
                    for m in range(2):
                        h = 2 * p + m
                        co = 64 * m
                        nc.tensor.matmul(
                            pav[p][co:co + 33, :],
                            vaug[kt][:, 33 * h:33 * h + 33],
                            att[p // 2][:, (p % 2) * 1024 + m * 512:
                                        (p % 2) * 1024 + m * 512 + 512],
                            start=(kt == 0), stop=(kt == KT - 1),
                            skip_group_check=True, tile_position=(0, co))

            # ---- normalize + out-projection ----
            # Engines need 32-aligned partition bases, so evacuate den rows
            # (psum rows 32/96 of 4 banks) at their own partitions into 4 col
            # blocks, then one SBUF->SBUF DMA gathers them to 8 partitions.
            # Gathered row order: p = (h%2)*4 + h//2 (row-major over (m, j)).
            denw = stage.tile([128, 4 * 512], FP32, name="denw", tag="denw")
            for h in range(8):
                j, m = h // 2, h % 2
                co = 64 * m + 32
                nc.scalar.copy(denw[co:co + 1, j * 512:(j + 1) * 512],
                               pav[j][co:co + 1, :])
            den = stage.tile([8, 512], FP32, name="den", tag="den")
            nc.sync.dma_start(out=den[0:4, :], in_=denw[32:33, :])
            nc.sync.dma_start(out=den[4:8, :], in_=denw[96:97, :])
            rec = stage.tile([8, 512], FP32, name="rec", tag="rec")
            nc.vector.reciprocal_approx_fast(out=rec, in_=den)
            rec16 = stage.tile([8, 512], FP16, name="rec16", tag="rec16")
            nc.scalar.copy(rec16, rec)
            Og = []
            bigE = psS.tile([128, 1024], FP32, name="endps", tag="scores")
            bigE2 = psS.tile([128, 1024], FP32, name="endps2", tag="scores")
            for g in range(2):
                rb = bigE[:, g * 512:g * 512 + 512]
                nc.tensor.matmul(rb, c_sb["ldn"][:, g * 128:(g + 1) * 128],
                                 rec16, start=True, stop=True)
                rbs = stage.tile([128, 512], FP32, name="rbs", tag="rbs")
                nc.scalar.copy(rbs, rb)
                og = stage.tile([128, 512], FP16, name=f"Og{g}", tag=f"Og{g}")
                for m in range(4):
                    h = 4 * g + m
                    nc.vector.tensor_tensor(
                        out=og[32 * m:32 * m + 32, :],
                        in0=pav[h // 2][64 * (h % 2):64 * (h % 2) + 32, :],
                        in1=rbs[32 * m:32 * m + 32, :], op=AOT.mult)
                Og.append(og)
            for go in range(2):
                ps = bigE2[:, go * 512:go * 512 + 512]
                for gi in range(2):
                    nc.tensor.matmul(
                        ps, w_sb["wp", gi][:, go * 128:(go + 1) * 128], Og[gi],
                        start=(gi == 0), stop=(gi == 1))
                fin = stage.tile([128, 512], FP32, name="fin", tag="fin")
                nc.scalar.activation(fin, ps, Ident, bias=c_sb["bps"][:, go:go + 1])
                nc.sync.dma_start(
                    out=outT[go * 128:(go + 1) * 128, q0:q0 + 512], in_=fin)


_CACHE = {}


def _get_kernel():
    if "nc" not in _CACHE:
        _CACHE["nc"] = _build()
    return _CACHE["nc"]


def prepare_in_maps(query, key, value, rel_pos, Wk, bk, Wv, bv, Wq, bq, Wp, bp,
                    emb_fwd, emb_bwd):
    query = np.asarray(query, dtype=np.float32)
    key = np.asarray(key, dtype=np.float32)
    value = np.asarray(value, dtype=np.float32)
    rel_pos = np.asarray(rel_pos, dtype=np.int32)
    Wk, Wv, Wq, Wp = (np.asarray(w, dtype=np.float32) for w in (Wk, Wv, Wq, Wp))
    bk, bv, bq, bp = (np.asarray(v, dtype=np.float32) for v in (bk, bv, bq, bp))
    emb_fwd = np.asarray(emb_fwd, dtype=np.float32)
    emb_bwd = np.asarray(emb_bwd, dtype=np.float32)

    # 100-entry bias LUT: T2[10*i+j, h] = ef[i,h] + eb[j,h]
    T2 = (emb_fwd[:, None, :] + emb_bwd[None, :, :]).reshape(100, H)
    s = float(max(np.abs(T2).max() / 127.0, 1e-6))
    lut = np.round(T2 / s).astype(np.int8)         # [100, H] (heads 0-5 used)
    lutT = np.ascontiguousarray(lut.T)             # [H, 100]
    lutFT = np.ascontiguousarray((T2.T / s).astype(np.float16))  # heads 6-7

    gamma = 1.0 / np.sqrt(np.float32(D_K))
    wqT = np.ascontiguousarray((Wq.T * (gamma / s)).astype(np.float16))
    wkT = np.ascontiguousarray(Wk.T.astype(np.float16))
    wvT = np.ascontiguousarray(Wv.T.astype(np.float16))
    wpT = np.ascontiguousarray(Wp.T.astype(np.float16))
    bqs = np.ascontiguousarray((bq * (gamma / s)).reshape(2, 128).T.astype(np.float32))
    # bk is softmax-invariant (adds a per-(h,q) constant across k); dropped.
    # bv folds into bp since softmax rows sum to 1.
    bps2 = bp.astype(np.float64) + Wp.astype(np.float64) @ bv.astype(np.float64)
    bps = np.ascontiguousarray(bps2.reshape(2, 128).T.astype(np.float32))
    # den rows arrive DMA-gathered in order p = (h%2)*4 + h//2
    ldnc = np.zeros((8, 256), np.float16)
    for h in range(H):
        g, m = h // 4, h % 4
        p = (h % 2) * 4 + h // 2
        ldnc[p, g * 128 + 32 * m: g * 128 + 32 * m + 32] = 1.0
    svec = np.stack([np.full(128, s, np.float32),
                     np.full(128, -4.0, np.float32)], axis=1)

    in_maps = []
    for core in range(N_CORES):
        b, half = divmod(core, 2)
        qs = half * QH
        rp = rel_pos[b]
        # bias[h,k,q] = ef[rp[qs+q,k],h] + eb[rp[k,qs+q],h] via LUT on
        # c[k,q] = 10*rp[qs+q,k] + rp[k,qs+q]
        c = rp[qs:qs + QH, :].T * 10 + rp[:, qs:qs + QH]
        bias_hkq = lutT[0:6][:, c]                 # [6, S, QH] int8
        biasF_hkq = lutFT[6:8][:, c]               # [2, S, QH] fp16
        # pack to DMA-tile order: [qc, kt, k(128), h, q(512)]
        bias_dev = np.ascontiguousarray(
            bias_hkq.reshape(6, KT, 128, 2, 512).transpose(3, 1, 2, 0, 4)
        ).reshape(32 * 128, 6 * 512)
        biasF_dev = np.ascontiguousarray(
            biasF_hkq.reshape(2, KT, 128, 2, 512).transpose(3, 1, 2, 0, 4)
        ).reshape(32 * 128, 2 * 512)
        m = {
            "qT": np.ascontiguousarray(query[b, qs:qs + QH, :].T.astype(np.float16)),
            "kT": np.ascontiguousarray(key[b].T.astype(np.float16)),
            "vT": np.ascontiguousarray(value[b].T.astype(np.float16)),
            "wqT": wqT, "wkT": wkT, "wvT": wvT, "wpT": wpT,
            "bqs": bqs, "bps": bps, "ldn": ldnc, "svec": svec,
            "biasT": bias_dev, "biasF": biasF_dev,
            "idt": np.eye(128, dtype=np.float16),
        }
        in_maps.append(m)
    return in_maps


def kernel(**inputs):
    nc = _get_kernel()
    in_maps = prepare_in_maps(**inputs)

    global LAST_IN_MAPS
    LAST_IN_MAPS = in_maps
    res = run_bass_kernel_spmd(nc, in_maps, list(range(N_CORES)))

    out = np.empty((B, S, D), dtype=np.float32)
    for core in range(N_CORES):
        b, half = divmod(core, 2)
        qs = half * QH
        out[b, qs:qs + QH, :] = res.results[core]["outT"].T
    return out


# revision 41
# speedup vs baseline: 1.0966x; 1.0966x over previous
"""Trainium2 Bass kernel v3: MultiHeadAttention with rel-pos bias via
host-LUT bias tiles + per-head score layout + PE array tiling.

Problem: B=4, S=2048, D=256, H=8, d_k=32.  8 cores = (batch, query-half);
each core: 8 heads x 1024 q x 2048 k.

v2 (one-hot-plane matmuls in a packed (h,k16) layout) was PE-bound
(~924us MATMUL: 8 matmuls of 512 free per 128x512 score tile) plus 75MB
of 9x-replicated rel_pos DMA (~104 GB/s achieved -> DMA co-critical).

v3: the bias bias[k,q,h] = ef[rpF,h] + eb[rpN,h] takes only 100 values
per head, so the host folds it through a 100x8 LUT into int8 tiles laid
out exactly as the SBUF tiles consume them (16MB/core, contiguous 512KB
DMAs).  Device uses a per-head score layout [128 k, 512 q]:
  scores = K_h^T Q_h / s        1 matmul, contraction 32, row-tiled:
                                4 heads run CONCURRENT in the PE array
                                (tile_position=(32m,0), 4 PSUM banks)
  att_pre = scores + q8         DVE add (int8 bias), out fp16 SBUF
  att     = exp(s*att_pre)      ACT, fp16 in/out (2x rate), scale=s
  psAV   += Vaug_h^T att        1 matmul; Vaug has a ones column so the
                                denominator rides along as out row 32;
                                2 heads/bank at offsets {0,64} run
                                concurrent via col tiling
All KQ packs are emitted before all AV pairs within a k-tile so the PE
stream has only 2 tiling-mode transitions per k-tile -- interleaving
row-mode and col-mode matmuls every ~2 instructions intermittently hard
-crashed an exec unit (NRT_EXEC_UNIT_UNRECOVERABLE, ~20% of runs).

Measured: 247us HW exec (vs 784us v2 baseline), rel err 0.0081.
Rejected variants: bias-add via PE identity matmul for 2/8 heads (269us:
the full-mode matmuls serialize against the row/col-tiled packs); 4-bank
score tile without double buffering (310us); fine-grained per-head ops
(350us, semaphore-bound).
"""

import sys

if "/opt/trn_rl_repo" not in sys.path:
    sys.path.insert(0, "/opt/trn_rl_repo")

import numpy as np

import concourse.bass as bass
import concourse.mybir as mybir
from concourse import bacc
from concourse.tile import TileContext
from concourse.bass_utils import run_bass_kernel_spmd

B, S, D, H = 4, 2048, 256, 8
D_K = D // H
QH = S // 2
N_CORES = 8
KT = S // 128           # 16 k-tiles of 128
FP32 = mybir.dt.float32
FP16 = mybir.dt.float16
BF16 = mybir.dt.bfloat16
INT8 = mybir.dt.int8

BIAS_INT8 = True        # True: int8 bias tiles (16MB/core); False: fp16 (32MB)
BIAS_DT = INT8 if BIAS_INT8 else FP16


def _build():
    nc = bacc.Bacc("TRN2", target_bir_lowering=False, debug=False)

    qT = nc.dram_tensor("qT", [D, QH], FP16, kind="ExternalInput").ap()
    kT = nc.dram_tensor("kT", [D, S], FP16, kind="ExternalInput").ap()
    vT = nc.dram_tensor("vT", [D, S], FP16, kind="ExternalInput").ap()
    wqT = nc.dram_tensor("wqT", [D, D], FP16, kind="ExternalInput").ap()
    wkT = nc.dram_tensor("wkT", [D, D], FP16, kind="ExternalInput").ap()
    wvT = nc.dram_tensor("wvT", [D, D], FP16, kind="ExternalInput").ap()
    wpT = nc.dram_tensor("wpT", [D, D], FP16, kind="ExternalInput").ap()
    bqs = nc.dram_tensor("bqs", [128, 2], FP32, kind="ExternalInput").ap()
    bps = nc.dram_tensor("bps", [128, 2], FP32, kind="ExternalInput").ap()
    ldn = nc.dram_tensor("ldn", [8, 256], FP16, kind="ExternalInput").ap()
    svec = nc.dram_tensor("svec", [128, 2], FP32, kind="ExternalInput").ap()
    # bias tiles pre-packed host-side: row block (qc*16+kt)*128 .. +128 is
    # one SBUF tile [128 k, (8 h, 512 q)]
    biasT = nc.dram_tensor("biasT", [32 * 128, 8 * 512], BIAS_DT,
                           kind="ExternalInput").ap()
    outT = nc.dram_tensor("outT", [D, QH], FP32, kind="ExternalOutput").ap()

    with TileContext(nc) as tc:
        _emit(nc, tc, locals())
    nc.compile()
    return nc


def _emit(nc, tc, t):
    qT, kT, vT = t["qT"], t["kT"], t["vT"]
    wqT, wkT, wvT, wpT = t["wqT"], t["wkT"], t["wvT"], t["wpT"]
    bqs, bps, ldn, svec = t["bqs"], t["bps"], t["ldn"], t["svec"]
    biasT, outT = t["biasT"], t["outT"]
    Exp = mybir.ActivationFunctionType.Exp
    Ident = mybir.ActivationFunctionType.Identity
    AOT = mybir.AluOpType

    import contextlib
    ctx = contextlib.ExitStack()
    with ctx:
        singles = ctx.enter_context(tc.tile_pool(name="singles", bufs=1))
        stage = ctx.enter_context(tc.tile_pool(name="stage", bufs=2))
        biasp = ctx.enter_context(tc.tile_pool(name="biasp", bufs=4))
        prep = ctx.enter_context(tc.tile_pool(name="prep", bufs=3))
        attp = ctx.enter_context(tc.tile_pool(name="attp", bufs=2))
        # two 2-bank score tiles [128, 1024] rotate so the PE fills one
        # while the DVE drains the other; all other psum users slice them
        psS = ctx.enter_context(tc.tile_pool(name="psS", bufs=2, space="PSUM"))
        psAV = ctx.enter_context(tc.tile_pool(name="psAV", bufs=1, space="PSUM"))

        # ---- constants ----
        c_sb = {}
        for name, ap, shp, dt in (
            ("bqs", bqs, [128, 2], FP32), ("bps", bps, [128, 2], FP32),
            ("ldn", ldn, [8, 256], FP16), ("svec", svec, [128, 2], FP32),
        ):
            tl = singles.tile(shp, dt, name=name, tag=name)
            nc.sync.dma_start(out=tl, in_=ap)
            c_sb[name] = tl

        # ---- weights: [din-group][128, 256] ----
        w_sb = {}
        for name, ap in (("wq", wqT), ("wk", wkT), ("wv", wvT), ("wp", wpT)):
            for g in range(2):
                tl = singles.tile([128, D], FP16, name=f"w_{name}{g}", tag=f"w_{name}{g}")
                nc.sync.dma_start(out=tl, in_=ap[g * 128:(g + 1) * 128, :])
                w_sb[name, g] = tl

        # ---- raw inputs resident ----
        xin = {}
        for name, ap, width in (("q", qT, QH), ("k", kT, S), ("v", vT, S)):
            for g in range(2):
                tl = singles.tile([128, width], FP16, name=f"{name}in{g}", tag=f"{name}in{g}")
                nc.sync.dma_start(out=tl, in_=ap[g * 128:(g + 1) * 128, :])
                xin[name, g] = tl

        # ---- Q/K projections -> QTs/KTs [g][128, *] fp16 (dout-major) ----
        QTs = [singles.tile([128, QH], FP16, name=f"QTs{g}", tag=f"QTs{g}") for g in range(2)]
        KTs = [singles.tile([128, S], FP16, name=f"KTs{g}", tag=f"KTs{g}") for g in range(2)]
        for dst, src, wname, bias_name, width in (
            (QTs, "q", "wq", "bqs", QH),
            (KTs, "k", "wk", None, S),
        ):
            for c0 in range(0, width, 512):
                big = psS.tile([128, 1024], FP32, name="proj", tag="scores")
                for g in range(2):
                    ps = big[:, g * 512:g * 512 + 512]
                    for dg in range(2):
                        nc.tensor.matmul(
                            ps, w_sb[wname, dg][:, g * 128:(g + 1) * 128],
                            xin[src, dg][:, c0:c0 + 512],
                            start=(dg == 0), stop=(dg == 1))
                    # alternate psum evacuation between ACT and DVE so the
                    # prologue isn't serialized on one engine
                    if bias_name:
                        if g == 0:
                            nc.scalar.activation(
                                dst[g][:, c0:c0 + 512], ps, Ident,
                                bias=c_sb[bias_name][:, g:g + 1])
                        else:
                            nc.vector.tensor_scalar(
                                out=dst[g][:, c0:c0 + 512], in0=ps,
                                scalar1=c_sb[bias_name][:, g:g + 1],
                                scalar2=None, op0=AOT.add)
                    else:
                        if g == 0:
                            nc.scalar.copy(dst[g][:, c0:c0 + 512], ps)
                        else:
                            nc.vector.tensor_scalar_add(
                                dst[g][:, c0:c0 + 512], ps, 0.0)

        # ---- Vaug[kt] [128 s, 264=(h: 32 dv + one)] fp16 ----
        vaug = []
        for kt2 in range(KT // 2):
            big = psS.tile([128, 1024], FP32, name="vproj", tag="scores")
            for ki in range(2):
                kt = kt2 * 2 + ki
                vt = singles.tile([128, 264], FP16, name=f"vaug{kt}", tag=f"vaug{kt}")
                ones_ap = bass.AP(tensor=vt.tensor, offset=vt.offset + 32,
                                  ap=[list(vt.ap[0]), [33, 8]])
                nc.gpsimd.memset(ones_ap, 1.0)
                vps = big[:, ki * 512:ki * 512 + 512]
                for dg in range(2):
                    nc.tensor.matmul(
                        vps[:, 0:256], xin["v", dg][:, kt * 128:(kt + 1) * 128],
                        w_sb["wv", dg], start=(dg == 0), stop=(dg == 1))
                dst_ap = bass.AP(tensor=vt.tensor, offset=vt.offset,
                                 ap=[list(vt.ap[0]), [33, 8], [1, 32]])
                src_ap = bass.AP(tensor=vps.tensor, offset=vps.offset,
                                 ap=[list(vps.ap[0]), [32, 8], [1, 32]])
                if ki == 0:
                    nc.scalar.copy(dst_ap, src_ap)
                else:
                    nc.vector.tensor_scalar_add(dst_ap, src_ap, 0.0)
                vaug.append(vt)

        # ---- main loop ----
        for qc in range(2):
            q0 = qc * 512
            pav = [psAV.tile([128, 512], FP32, name=f"psAV{j}", tag=f"psAV{j}")
                   for j in range(4)]
            for kt in range(KT):
                bt = biasp.tile([128, 8 * 512], BIAS_DT, name="bt", tag="bt")
                r0 = (qc * KT + kt) * 128
                for quad in range(4):
                    eng = (nc.sync, nc.gpsimd)[quad % 2]
                    eng.dma_start(
                        out=bt[:, quad * 1024:(quad + 1) * 1024],
                        in_=biasT[r0:r0 + 128, quad * 1024:(quad + 1) * 1024])
                # per head-pair p: 2 row-packed concurrent KQ matmuls into the
                # bank slices of a double-buffered 2-bank psum tile, then ONE
                # DVE add and ONE ACT exp over [128, 1024]
                # Emit all KQ packs first, all AV pairs last: the PE stream
                # then has only 2 tiling-mode transitions (row<->col) per kt.
                # DVE adds run per 2-bank psum tile [128,1024]; exp merges two
                # adds into one [128,2048] ACT op (halves ACT op overhead).
                att = []
                apre = None
                for p in range(4):
                    g = p // 2
                    big = psS.tile([128, 1024], FP32, name="scores", tag="scores")
                    for m in range(2):
                        r = 32 * ((2 * p + m) % 4)
                        nc.tensor.matmul(
                            big[:, m * 512:m * 512 + 512],
                            KTs[g][r:r + 32, kt * 128:(kt + 1) * 128],
                            QTs[g][r:r + 32, q0:q0 + 512], start=True, stop=True,
                            tile_position=(r, 0))
                    if p % 2 == 0:
                        apre = attp.tile([128, 2048], FP16, name="att_pre",
                                         tag="att_pre")
                    hp = (p % 2) * 1024
                    nc.vector.tensor_tensor(
                        out=apre[:, hp:hp + 1024], in0=big,
                        in1=bt[:, p * 1024:(p + 1) * 1024], op=AOT.add)
                    if p % 2 == 1:
                        # -4 shift keeps exp in fp16 range (logit tail ~12.5 >
                        # ln 65504); cancels between num. and denominator.
                        at = prep.tile([128, 2048], FP16, name="att", tag="att")
                        nc.scalar.activation(at, apre, Exp,
                                             bias=c_sb["svec"][:, 1:2],
                                             scale=c_sb["svec"][:, 0:1])
                        att.append(at)
                # AV+den: col-packed pairs, 2 heads into bank p at {0, 64}
                for p in range(4):
                    for m in range(2):
                        h = 2 * p + m
                        co = 64 * m
                        nc.tensor.matmul(
                            pav[p][co:co + 33, :],
                            vaug[kt][:, 33 * h:33 * h + 33],
                            att[p // 2][:, (p % 2) * 1024 + m * 512:
                                        (p % 2) * 1024 + m * 512 + 512],
                            start=(kt == 0), stop=(kt == KT - 1),
                            skip_group_check=True, tile_position=(0, co))

            # ---- normalize + out-projection ----
            # Engines need 32-aligned partition bases, so evacuate den rows
            # (psum rows 32/96 of 4 banks) at their own partitions into 4 col
            # blocks, then one SBUF->SBUF DMA gathers them to 8 partitions.
            # Gathered row order: p = (h%2)*4 + h//2 (row-major over (m, j)).
            denw = stage.tile([128, 4 * 512], FP32, name="denw", tag="denw")
            for h in range(8):
                j, m = h // 2, h % 2
                co = 64 * m + 32
                nc.scalar.copy(denw[co:co + 1, j * 512:(j + 1) * 512],
                               pav[j][co:co + 1, :])
            den = stage.tile([8, 512], FP32, name="den", tag="den")
            nc.sync.dma_start(out=den[0:4, :], in_=denw[32:33, :])
            nc.sync.dma_start(out=den[4:8, :], in_=denw[96:97, :])
            rec = stage.tile([8, 512], FP32, name="rec", tag="rec")
            nc.vector.reciprocal_approx_fast(out=rec, in_=den)
            rec16 = stage.tile([8, 512], FP16, name="rec16", tag="rec16")
            nc.scalar.copy(rec16, rec)
            Og = []
            bigE = psS.tile([128, 1024], FP32, name="endps", tag="scores")
            bigE2 = psS.tile([128, 1024], FP32, name="endps2", tag="scores")
            for g in range(2):
                rb = bigE[:, g * 512:g * 512 + 512]
                nc.tensor.matmul(rb, c_sb["ldn"][:, g * 128:(g + 1) * 128],
                                 rec16, start=True, stop=True)
                rbs = stage.tile([128, 512], FP32, name="rbs", tag="rbs")
                nc.scalar.copy(rbs, rb)
                og = stage.tile([128, 512], FP16, name=f"Og{g}", tag=f"Og{g}")
                for m in range(4):
                    h = 4 * g + m
                    nc.vector.tensor_tensor(
                        out=og[32 * m:32 * m + 32, :],
                        in0=pav[h // 2][64 * (h % 2):64 * (h % 2) + 32, :],
                        in1=rbs[32 * m:32 * m + 32, :], op=AOT.mult)
                Og.append(og)
            for go in range(2):
                ps = bigE2[:, go * 512:go * 512 + 512]
                for gi in range(2):
                    nc.tensor.matmul(
                        ps, w_sb["wp", gi][:, go * 128:(go + 1) * 128], Og[gi],
                        start=(gi == 0), stop=(gi == 1))
                fin = stage.tile([128, 512], FP32, name="fin", tag="fin")
                nc.scalar.activation(fin, ps, Ident, bias=c_sb["bps"][:, go:go + 1])
                nc.sync.dma_start(
                    out=outT[go * 128:(go + 1) * 128, q0:q0 + 512], in_=fin)


_CACHE = {}


def _get_kernel():
    if "nc" not in _CACHE:
        _CACHE["nc"] = _build()
    return _CACHE["nc"]


def prepare_in_maps(query, key, value, rel_pos, Wk, bk, Wv, bv, Wq, bq, Wp, bp,
                    emb_fwd, emb_bwd):
    query = np.asarray(query, dtype=np.float32)
    key = np.asarray(key, dtype=np.float32)
    value = np.asarray(value, dtype=np.float32)
    rel_pos = np.asarray(rel_pos, dtype=np.int32)
    Wk, Wv, Wq, Wp = (np.asarray(w, dtype=np.float32) for w in (Wk, Wv, Wq, Wp))
    bk, bv, bq, bp = (np.asarray(v, dtype=np.float32) for v in (bk, bv, bq, bp))
    emb_fwd = np.asarray(emb_fwd, dtype=np.float32)
    emb_bwd = np.asarray(emb_bwd, dtype=np.float32)

    # 100-entry bias LUT: T2[10*i+j, h] = ef[i,h] + eb[j,h]
    T2 = (emb_fwd[:, None, :] + emb_bwd[None, :, :]).reshape(100, H)
    if BIAS_INT8:
        s = float(max(np.abs(T2).max() / 127.0, 1e-6))
        lut = np.round(T2 / s).astype(np.int8)     # [100, H]
    else:
        s = 1.0
        lut = T2.astype(np.float16)
    lutT = np.ascontiguousarray(lut.T)             # [H, 100]

    gamma = 1.0 / np.sqrt(np.float32(D_K))
    wqT = np.ascontiguousarray((Wq.T * (gamma / s)).astype(np.float16))
    wkT = np.ascontiguousarray(Wk.T.astype(np.float16))
    wvT = np.ascontiguousarray(Wv.T.astype(np.float16))
    wpT = np.ascontiguousarray(Wp.T.astype(np.float16))
    bqs = np.ascontiguousarray((bq * (gamma / s)).reshape(2, 128).T.astype(np.float32))
    # bk is softmax-invariant (adds a per-(h,q) constant across k); dropped.
    # bv folds into bp since softmax rows sum to 1.
    bps2 = bp.astype(np.float64) + Wp.astype(np.float64) @ bv.astype(np.float64)
    bps = np.ascontiguousarray(bps2.reshape(2, 128).T.astype(np.float32))
    # den rows arrive DMA-gathered in order p = (h%2)*4 + h//2
    ldnc = np.zeros((8, 256), np.float16)
    for h in range(H):
        g, m = h // 4, h % 4
        p = (h % 2) * 4 + h // 2
        ldnc[p, g * 128 + 32 * m: g * 128 + 32 * m + 32] = 1.0
    svec = np.stack([np.full(128, s, np.float32),
                     np.full(128, -4.0, np.float32)], axis=1)

    in_maps = []
    for core in range(N_CORES):
        b, half = divmod(core, 2)
        qs = half * QH
        rp = rel_pos[b]
        # bias[h,k,q] = ef[rp[qs+q,k],h] + eb[rp[k,qs+q],h] via LUT on
        # c[k,q] = 10*rp[qs+q,k] + rp[k,qs+q]
        c = rp[qs:qs + QH, :].T * 10 + rp[:, qs:qs + QH]
        bias_hkq = lutT[:, c]                      # [H, S, QH]
        # pack to DMA-tile order: [qc, kt, k(128), h, q(512)]
        bias_dev = np.ascontiguousarray(
            bias_hkq.reshape(H, KT, 128, 2, 512).transpose(3, 1, 2, 0, 4)
        ).reshape(32 * 128, 8 * 512)
        m = {
            "qT": np.ascontiguousarray(query[b, qs:qs + QH, :].T.astype(np.float16)),
            "kT": np.ascontiguousarray(key[b].T.astype(np.float16)),
            "vT": np.ascontiguousarray(value[b].T.astype(np.float16)),
            "wqT": wqT, "wkT": wkT, "wvT": wvT, "wpT": wpT,
            "bqs": bqs, "bps": bps, "ldn": ldnc, "svec": svec,
            "biasT": bias_dev,
        }
        in_maps.append(m)
    return in_maps


def kernel(**inputs):
    nc = _get_kernel()
    in_maps = prepare_in_maps(**inputs)

    global LAST_IN_MAPS
    LAST_IN_MAPS = in_maps
    res = run_bass_kernel_spmd(nc, in_maps, list(range(N_CORES)))

    out = np.empty((B, S, D), dtype=np.float32)
    for core in range(N_CORES):
        b, half = divmod(core, 2)
        qs = half * QH
        out[b, qs:qs + QH, :] = res.results[core]["outT"].T
    return out
